# revision 9
# baseline (speedup 1.0000x reference)
"""Trainium2 Bass kernel: multi-head attention (dense transformer block).

Computation (per batch b):
    Q = x @ Wq + bq ; K = x @ Wk + bk ; V = x @ Wv + bv        (per head)
    P = exp((Q @ K^T) / sqrt(Dh))                               (no max-sub; scores are O(1))
    out = sum_h (P @ V / rowsum(P)) @ Wd[h] + bd

Sharding: 8 cores; core c handles batch b = c // 4 and 4 heads h0 = 4*(c%4).
Each core returns a partial [L, D] output; host sums groups of 4 cores + bd.

Per-core dataflow (SPMD program, all cores identical):
  - x^T built on-chip via PE transposes (f32).
  - Heads processed in 2 pairs; QKV projections computed as Q^T/K^T [128(2 heads
    stacked on partitions), L] so scores S^T = K^T.T @ Q^T land with softmax axis
    on PSUM partitions; exp on ScalarE (scale=1/8 fused) -> P^T bf16.
  - attend: O^T = [V|1].T @ P^T; the appended ones-column makes PSUM row 64 the
    softmax denominator for free.
  - normalize: recip (DVE) -> broadcast via K-padded PE outer product -> mul.
  - out-proj: Y = O^T.T @ Wd accumulated over head pairs in PSUM (f32r).
Matmuls in the scores path use float32r (full fp32 data, fast PE path).
"""

import os
import sys
from contextlib import ExitStack

import numpy as np

for _p in ("/opt/trn_rl_repo", "/root/.axon_site/_ro/trn_rl_repo"):
    if os.path.isdir(_p) and _p not in sys.path:
        sys.path.append(_p)

import concourse.bass as bass
import concourse.tile as tile
from concourse import bacc, mybir
from concourse.bass import ds, ts
from concourse.bass_utils import run_bass_kernel_spmd
from concourse.masks import make_identity
from concourse.tile_rust import add_dep_helper

F32 = mybir.dt.float32
F32R = mybir.dt.float32r
BF16 = mybir.dt.bfloat16

# Problem sizes (hardcoded per contract).
DMODEL, HEADS, DHEAD = 1024, 16, 64
B, L = 2, 2048
NCORES = 8
H_PER_CORE = B * HEADS // NCORES          # 4 heads per core
NPAIR = H_PER_CORE // 2                   # head pairs per core
P = 128                                   # partitions
KT = DMODEL // P                          # 8 k-tiles over dmodel
NLT = L // P                              # 16 l-tiles
LCH = 512                                 # l-chunk (psum free dim)
NLC = L // LCH                            # 4 l-chunks
MCH = 512                                 # m-chunk for out-proj
NMC = DMODEL // MCH

DT_ATT = BF16                             # P^T / V / attend operand dtype


def _r(ap):
    """fp32 tile viewed as float32r for the fast PE path."""
    return ap.bitcast(F32R)


def build_nc(interleave_attend=True):
    """Build the SPMD Bass program for one core."""
    nc = bacc.Bacc("TRN2", target_bir_lowering=False, debug=False,
                   num_devices=NCORES)

    x_d = nc.dram_tensor("x", [L, DMODEL], F32, kind="ExternalInput").ap()
    wq_d = nc.dram_tensor("wq", [DMODEL, H_PER_CORE * DHEAD], F32R, kind="ExternalInput").ap()
    wk_d = nc.dram_tensor("wk", [DMODEL, H_PER_CORE * DHEAD], F32R, kind="ExternalInput").ap()
    wv_d = nc.dram_tensor("wv", [DMODEL, H_PER_CORE * DHEAD], F32R, kind="ExternalInput").ap()
    wd_d = nc.dram_tensor("wd", [H_PER_CORE * DHEAD, DMODEL], F32R, kind="ExternalInput").ap()
    bq_d = nc.dram_tensor("bq", [H_PER_CORE * DHEAD], F32, kind="ExternalInput").ap()
    bk_d = nc.dram_tensor("bk", [H_PER_CORE * DHEAD], F32, kind="ExternalInput").ap()
    bv_d = nc.dram_tensor("bv", [H_PER_CORE * DHEAD], F32, kind="ExternalInput").ap()
    y_d = nc.dram_tensor("y", [L, DMODEL], F32, kind="ExternalOutput").ap()

    with TileKernel(nc) as tk:
        tk.body(x_d, wq_d, wk_d, wv_d, wd_d, bq_d, bk_d, bv_d, y_d,
                interleave_attend=interleave_attend)
    nc.compile()
    return nc


class TileKernel:
    def __init__(self, nc):
        self.nc = nc
        self.ctx = ExitStack()
        self.tc = None

    def __enter__(self):
        self.tc = self.ctx.enter_context(tile.TileContext(self.nc))
        return self

    def __exit__(self, *exc):
        return self.ctx.__exit__(*exc)

    def body(self, x_d, wq_d, wk_d, wv_d, wd_d, bq_d, bk_d, bv_d, y_d,
             interleave_attend=True):
        nc, tc, ctx = self.nc, self.tc, self.ctx

        const = ctx.enter_context(tc.tile_pool(name="const", bufs=1))
        sb = ctx.enter_context(tc.tile_pool(name="sb", bufs=1))
        psum = ctx.enter_context(tc.tile_pool(name="psum", bufs=1, space="PSUM"))

        ident = const.tile([P, P], F32)
        make_identity(nc, ident)
        ident_r = const.tile([P, P], F32R)
        nc.vector.tensor_copy(ident_r, ident)
        # f32r tiles can't be memset directly; round-copy from f32 sources.
        zsrc = const.tile([P, LCH], F32)
        nc.vector.memset(zsrc, 0.0)
        ones = const.tile([P, P], F32R)
        osrc = const.tile([P, P], F32)
        nc.vector.memset(osrc, 1.0)
        nc.vector.tensor_copy(ones, osrc)
        # rows 1..127 stay zero forever; row 0 gets recip denominators.
        rd0 = const.tile([P, LCH], F32R)
        nc.vector.tensor_copy(rd0, zsrc)

        # biases: [p*128 + i] layout matches head-pair partition stacking.
        bias_sb = const.tile([P, 3, NPAIR], F32)
        for i, b_d in enumerate((bq_d, bk_d, bv_d)):
            for p in range(NPAIR):
                nc.sync.dma_start(bias_sb[:, i, p:p + 1],
                                  b_d.rearrange("(a p) -> a p", p=P)[p:p + 1, :]
                                  .rearrange("a p -> p a"))

        # ---- phase 0: x^T ----
        xt = sb.tile([P, KT, L], F32R)
        for lt in range(NLT):
            xs = sb.tile([P, DMODEL], F32, tag="xstage", bufs=2)
            nc.sync.dma_start(xs, x_d[ds(lt * P, P), :])
            for kt in range(KT):
                tp = psum.tile([P, P], F32, tag="trp", bufs=2)
                nc.tensor.transpose(tp, xs[:, ds(kt * P, P)], ident)
                nc.vector.tensor_copy(xt[:, kt, ds(lt * P, P)], tp)

        # out-proj weights, full [128(2h stacked), pair, D] f32
        wd_sb = const.tile([P, NPAIR, DMODEL], F32R)
        nc.sync.dma_start(
            wd_sb, wd_d.rearrange("(pp k) m -> k pp m", k=P))

        o_norm = sb.tile([P, NPAIR, L], F32R)

        prev_block_last = [None]  # last PE inst of previous scores/attend block

        for p in range(NPAIR):
            # ---- pair weights ----
            w_sb = sb.tile([P, 3, KT, P], F32R, tag="wqkv", bufs=1)
            for i, w_d in enumerate((wq_d, wk_d, wv_d)):
                nc.sync.dma_start(
                    w_sb[:, i],
                    w_d.rearrange("(kt k) m -> k kt m", k=P)[:, :, ds(p * P, P)])

            # ---- QKV projections (Q^T/K^T/V^T: [128 = 2 heads x 64, L]) ----
            qkv = []
            for i in range(3):
                dst = sb.tile([P, L], F32R, tag=f"qkv{i}", bufs=1)
                for lc in range(NLC):
                    ps = psum.tile([P, LCH], F32, tag="qkvp", bufs=2)
                    for kt in range(KT):
                        nc.tensor.matmul(
                            ps, lhsT=w_sb[:, i, kt],
                            rhs=xt[:, kt, ds(lc * LCH, LCH)],
                            start=(kt == 0), stop=(kt == KT - 1))
                    nc.vector.tensor_scalar_add(
                        dst[:, ds(lc * LCH, LCH)], ps, bias_sb[:, i, p:p + 1])
                qkv.append(dst)
            qT, kT_sb, vT = qkv

            # ---- V in [l', d] layout with ones columns: [128, lt, 130] ----
            vt = sb.tile([P, NLT, 2 * DHEAD + 2], DT_ATT, tag="vt", bufs=1)
            nc.vector.memset(vt[:, :, DHEAD:DHEAD + 1], 1.0)
            nc.vector.memset(vt[:, :, 2 * DHEAD + 1:2 * DHEAD + 2], 1.0)
            for lt in range(NLT):
                tp = psum.tile([P, P], F32R, tag="trp", bufs=2)
                nc.tensor.transpose(tp, vT[:, ds(lt * P, P)], ident_r)
                nc.vector.tensor_copy(vt[:, lt, 0:DHEAD], tp[:, 0:DHEAD])
                nc.vector.tensor_copy(
                    vt[:, lt, DHEAD + 1:2 * DHEAD + 1], tp[:, DHEAD:2 * DHEAD])

            # ---- scores -> exp -> attend -> normalize, per l-chunk ----
            for lc in range(NLC):
                sc_insts = []
                pt_tiles = [[None] * NLT, [None] * NLT]
                for lt in range(NLT):
                    for h in range(2):
                        sp = psum.tile([P, LCH], F32, tag="sc", bufs=2)
                        mm = nc.tensor.matmul(
                            sp,
                            lhsT=kT_sb[ds(64 * h, 64), ds(lt * P, P)],
                            rhs=qT[ds(64 * h, 64), ds(lc * LCH, LCH)],
                            start=True, stop=True)
                        sc_insts.append(mm)
                        pt = sb.tile([P, LCH], DT_ATT, tag="pt", bufs=36)
                        nc.scalar.activation(
                            pt, sp, func=mybir.ActivationFunctionType.Exp,
                            scale=1.0 / np.sqrt(DHEAD))
                        pt_tiles[h][lt] = pt
                if prev_block_last[0] is not None:
                    # keep 64-row-mode scores block after previous 128-mode block
                    add_dep_helper(prev_block_last[0].ins, sc_insts[0].ins,
                                   sync=False, reason="pe mode grouping")

                at_last = None
                for h in range(2):
                    op = psum.tile([P, LCH], F32, tag="op", bufs=1)
                    for lt in range(NLT):
                        mm = nc.tensor.matmul(
                            op[0:DHEAD + 1, :],
                            lhsT=vt[:, lt, ds((DHEAD + 1) * h, DHEAD + 1)],
                            rhs=pt_tiles[h][lt],
                            start=(lt == 0), stop=(lt == NLT - 1))
                        if lt == 0:
                            add_dep_helper(sc_insts[-1].ins, mm.ins, sync=False,
                                           reason="pe mode grouping")
                        at_last = mm
                    # normalize: rd0 row0 = 1/denom ; R = ones^T row0 x rd0
                    with nc.allow_low_precision(reason="f32r rounding of softmax denom recip"):
                        nc.vector.reciprocal(rd0[0:1, :], op[DHEAD:DHEAD + 1, :])
                    rp = psum.tile([P, LCH], F32, tag="rp", bufs=1)
                    nc.tensor.matmul(rp[0:DHEAD + 1, :], lhsT=ones[:, 0:DHEAD + 1],
                                     rhs=rd0, start=True, stop=True)
                    rs = sb.tile([DHEAD, LCH], F32, tag="rs", bufs=2)
                    nc.vector.tensor_copy(rs, rp[0:DHEAD, :])
                    nc.vector.tensor_mul(
                        o_norm[ds(64 * h, 64), p, ds(lc * LCH, LCH)],
                        op[0:DHEAD, :], rs)
                prev_block_last[0] = at_last

        # ---- out-projection: Y[l, m] = sum_pairs O^T.T @ Wd ----
        for lt in range(NLT):
            for mc in range(NMC):
                yp = psum.tile([P, MCH], F32, tag="qkvp", bufs=2)
                for p in range(NPAIR):
                    nc.tensor.matmul(
                        yp, lhsT=o_norm[:, p, ds(lt * P, P)],
                        rhs=wd_sb[:, p, ds(mc * MCH, MCH)],
                        start=(p == 0), stop=(p == NPAIR - 1))
                ys = sb.tile([P, MCH], F32, tag="ys", bufs=3)
                nc.vector.tensor_copy(ys, yp)
                nc.sync.dma_start(y_d[ds(lt * P, P), ds(mc * MCH, MCH)], ys)


_NC_CACHE = {}


def _get_nc():
    if "nc" not in _NC_CACHE:
        _NC_CACHE["nc"] = build_nc()
    return _NC_CACHE["nc"]


def shard_inputs(x, Wq, bq, Wk, bk, Wv, bv, Wd, bd):
    """Build the 8 per-core input maps."""
    in_maps = []
    for c in range(NCORES):
        b = c // (NCORES // B)
        h0 = (c % (NCORES // B)) * H_PER_CORE
        hs = slice(h0, h0 + H_PER_CORE)
        in_maps.append({
            "x": np.ascontiguousarray(np.asarray(x[b], np.float32)),
            "wq": np.ascontiguousarray(np.asarray(Wq[:, hs, :], np.float32).reshape(DMODEL, -1)),
            "wk": np.ascontiguousarray(np.asarray(Wk[:, hs, :], np.float32).reshape(DMODEL, -1)),
            "wv": np.ascontiguousarray(np.asarray(Wv[:, hs, :], np.float32).reshape(DMODEL, -1)),
            "wd": np.ascontiguousarray(np.asarray(Wd[hs], np.float32).reshape(-1, DMODEL)),
            "bq": np.ascontiguousarray(np.asarray(bq[hs], np.float32).reshape(-1)),
            "bk": np.ascontiguousarray(np.asarray(bk[hs], np.float32).reshape(-1)),
            "bv": np.ascontiguousarray(np.asarray(bv[hs], np.float32).reshape(-1)),
        })
    return in_maps


def gather_outputs(results, bd):
    """Sum partial outputs per batch and add bd."""
    out = np.zeros((B, L, DMODEL), np.float32)
    per_b = NCORES // B
    for c, res in enumerate(results):
        out[c // per_b] += res["y"]
    out += np.asarray(bd, np.float32)[None, None, :]
    return out


def kernel(x, Wq, bq, Wk, bk, Wv, bv, Wd, bd, _trace=False):
    nc = _get_nc()
    in_maps = shard_inputs(x, Wq, bq, Wk, bk, Wv, bv, Wd, bd)
    res = run_bass_kernel_spmd(nc, in_maps, list(range(NCORES)), trace=_trace)
    out = gather_outputs(res.results, bd)
    if _trace:
        kernel.last_results = res
    return out


# revision 13
# speedup vs baseline: 1.2893x; 1.2893x over previous
"""Trainium2 Bass kernel: multi-head attention (dense transformer block).

Computation (per batch b):
    Q = x @ Wq + bq ; K = x @ Wk + bk ; V = x @ Wv + bv        (per head)
    P = exp((Q @ K^T) / sqrt(Dh))                               (no max-sub; scores are O(1))
    out = sum_h (P @ V / rowsum(P)) @ Wd[h] + bd

Sharding: 8 cores; core c handles batch b = c // 4 and 4 heads h0 = 4*(c%4).
Each core returns a partial [L, D] output; host sums groups of 4 cores + bd.

Per-core dataflow (SPMD program, all cores identical):
  - x^T built on-chip via PE transposes (f32 data, fp32r matmul path).
  - Heads processed in 2 pairs; QKV projections (fp32r) computed as Q^T/K^T
    [128 = 2 heads stacked on partitions, L], drained to bf16, so scores
    S^T = K^T.T @ Q^T (bf16, fast-weight-load) land with the softmax axis on
    PSUM partitions; exp on ScalarE (scale fused, 1024-wide) -> P^T bf16.
  - attend: O^T = [V_h | ones*64].T @ P^T — the 64 replicated ones columns
    make PSUM rows 64..127 the softmax denominator, already broadcast, for
    free; normalize = recip + mul on DVE, no cross-partition ops needed.
  - out-proj: Y = O^T.T @ Wd accumulated over head pairs in PSUM (fp32r).
"""

import os
import sys
from contextlib import ExitStack

import numpy as np

for _p in ("/opt/trn_rl_repo", "/root/.axon_site/_ro/trn_rl_repo"):
    if os.path.isdir(_p) and _p not in sys.path:
        sys.path.append(_p)

import concourse.bass as bass
import concourse.tile as tile
from concourse import bacc, mybir
from concourse.bass import ds, ts
from concourse.bass_utils import run_bass_kernel_spmd
from concourse.masks import make_identity
from concourse.tile_rust import add_dep_helper

F32 = mybir.dt.float32
F32R = mybir.dt.float32r
BF16 = mybir.dt.bfloat16

# Problem sizes (hardcoded per contract).
DMODEL, HEADS, DHEAD = 1024, 16, 64
B, L = 2, 2048
NCORES = 8
H_PER_CORE = B * HEADS // NCORES          # 4 heads per core
NPAIR = H_PER_CORE // 2                   # head pairs per core
P = 128                                   # partitions
KT = DMODEL // P                          # 8 k-tiles over dmodel
NLT = L // P                              # 16 l-tiles
LCH = 512                                 # matmul free-dim chunk (one psum bank)
ECH = 1024                                # exp chunk (2 psum banks)
NEC = L // ECH                            # 2 exp chunks
MCH = 512                                 # m-chunk for out-proj
NMC = DMODEL // MCH

BLOCK_DEPS = os.environ.get("ATT_BLOCK_DEPS", "1") == "1"


def build_nc():
    """Build the SPMD Bass program for one core."""
    nc = bacc.Bacc("TRN2", target_bir_lowering=False, debug=False,
                   num_devices=NCORES)

    x_d = nc.dram_tensor("x", [L, DMODEL], F32, kind="ExternalInput").ap()
    wq_d = nc.dram_tensor("wq", [DMODEL, H_PER_CORE * DHEAD], F32R, kind="ExternalInput").ap()
    wk_d = nc.dram_tensor("wk", [DMODEL, H_PER_CORE * DHEAD], F32R, kind="ExternalInput").ap()
    wv_d = nc.dram_tensor("wv", [DMODEL, H_PER_CORE * DHEAD], F32R, kind="ExternalInput").ap()
    wd_d = nc.dram_tensor("wd", [H_PER_CORE * DHEAD, DMODEL], F32R, kind="ExternalInput").ap()
    bq_d = nc.dram_tensor("bq", [H_PER_CORE * DHEAD], F32, kind="ExternalInput").ap()
    bk_d = nc.dram_tensor("bk", [H_PER_CORE * DHEAD], F32, kind="ExternalInput").ap()
    bv_d = nc.dram_tensor("bv", [H_PER_CORE * DHEAD], F32, kind="ExternalInput").ap()
    y_d = nc.dram_tensor("y", [L, DMODEL], F32, kind="ExternalOutput").ap()

    with ExitStack() as ctx:
        tc = ctx.enter_context(tile.TileContext(nc))
        _body(nc, tc, ctx, x_d, wq_d, wk_d, wv_d, wd_d, bq_d, bk_d, bv_d, y_d)
    nc.compile()
    return nc


def _body(nc, tc, ctx, x_d, wq_d, wk_d, wv_d, wd_d, bq_d, bk_d, bv_d, y_d):
    const = ctx.enter_context(tc.tile_pool(name="const", bufs=1))
    sb = ctx.enter_context(tc.tile_pool(name="sb", bufs=1))
    psum = ctx.enter_context(tc.tile_pool(name="psum", bufs=1, space="PSUM"))

    ident = const.tile([P, P], F32)
    make_identity(nc, ident)
    ident_r = const.tile([P, P], F32R)
    nc.vector.tensor_copy(ident_r, ident)

    # biases: [pair*128 + i] layout matches head-pair partition stacking.
    bias_sb = const.tile([P, 3, NPAIR], F32)
    for i, b_d in enumerate((bq_d, bk_d, bv_d)):
        for p in range(NPAIR):
            nc.sync.dma_start(bias_sb[:, i, p:p + 1],
                              b_d.rearrange("(a p) -> a p", p=P)[p:p + 1, :]
                              .rearrange("a p -> p a"))

    # shared [128,1024] f32 psum slots: transposes AND scores use these
    def sc_tile():
        return psum.tile([P, ECH], F32, tag="sctr", bufs=2, name="sctr")

    # ---- phase 0: x^T ----
    xt = sb.tile([P, KT, L], F32R)
    for lt in range(NLT):
        xs = sb.tile([P, DMODEL], F32, tag="xstage", bufs=2)
        nc.sync.dma_start(xs, x_d[ds(lt * P, P), :])
        for kt in range(KT):
            tp = sc_tile()
            nc.tensor.transpose(tp[:, 0:P].bitcast(F32), xs[:, ds(kt * P, P)], ident)
            nc.vector.tensor_copy(xt[:, kt, ds(lt * P, P)], tp[:, 0:P])

    # out-proj weights, full [128(2h stacked), pair, D] f32r
    wd_sb = const.tile([P, NPAIR, DMODEL], F32R)
    nc.sync.dma_start(wd_sb, wd_d.rearrange("(pp k) m -> k pp m", k=P))

    o_norm = sb.tile([P, NPAIR, L], F32R)

    prev_block_last = [None]  # last PE inst of previous scores/attend block

    for p in range(NPAIR):
        # ---- pair weights ----
        w_sb = sb.tile([P, 3, KT, P], F32R, tag="wqkv", bufs=1)
        for i, w_d in enumerate((wq_d, wk_d, wv_d)):
            nc.sync.dma_start(
                w_sb[:, i],
                w_d.rearrange("(kt k) m -> k kt m", k=P)[:, :, ds(p * P, P)])

        # ---- QKV projections (Q^T/K^T/V^T: [128 = 2 heads x 64, L]) ----
        qkv = []
        for i in range(3):
            dt = F32 if i == 2 else BF16   # V^T stays f32 for its PE transpose
            dst = sb.tile([P, L], dt, tag=f"qkv{i}", bufs=1)
            for lc in range(L // LCH):
                ps = psum.tile([P, LCH], F32, tag="qkvp", bufs=2)
                for kt in range(KT):
                    nc.tensor.matmul(
                        ps, lhsT=w_sb[:, i, kt],
                        rhs=xt[:, kt, ds(lc * LCH, LCH)],
                        start=(kt == 0), stop=(kt == KT - 1))
                nc.vector.tensor_scalar_add(
                    dst[:, ds(lc * LCH, LCH)], ps, bias_sb[:, i, p:p + 1])
            qkv.append(dst)
        qT, kT_sb, vT = qkv

        # ---- V in [l', d] layout: per head [V_h (64) | ones (64)] bf16 ----
        vt = sb.tile([P, NLT, 2 * P], BF16, tag="vt", bufs=1)
        nc.vector.memset(vt[:, :, DHEAD:P], 1.0)
        nc.vector.memset(vt[:, :, P + DHEAD:2 * P], 1.0)
        for lt in range(NLT):
            tp = sc_tile()
            nc.tensor.transpose(tp[:, 0:P].bitcast(F32), vT[:, ds(lt * P, P)], ident)
            nc.vector.tensor_copy(vt[:, lt, 0:DHEAD], tp[:, 0:DHEAD])
            nc.vector.tensor_copy(vt[:, lt, P:P + DHEAD], tp[:, DHEAD:2 * DHEAD])

        # ---- scores -> exp -> attend -> normalize, per (exp-chunk, head) ----
        for ec in range(NEC):
            for h in range(2):
                sc_insts = []
                pt_tiles = [None] * NLT
                for lt in range(NLT):
                    sp = sc_tile()
                    for sub in range(ECH // LCH):
                        mm = nc.tensor.matmul(
                            sp[:, ds(sub * LCH, LCH)],
                            lhsT=kT_sb[ds(64 * h, 64), ds(lt * P, P)],
                            rhs=qT[ds(64 * h, 64),
                                   ds(ec * ECH + sub * LCH, LCH)],
                            start=True, stop=True)
                        sc_insts.append(mm)
                    pt = sb.tile([P, ECH], BF16, tag="pt", bufs=20)
                    nc.scalar.activation(
                        pt, sp, func=mybir.ActivationFunctionType.Exp,
                        scale=1.0 / np.sqrt(DHEAD))
                    pt_tiles[lt] = pt
                if BLOCK_DEPS and prev_block_last[0] is not None:
                    for mm in sc_insts:
                        add_dep_helper(prev_block_last[0].ins, mm.ins,
                                       sync=False, reason="pe mode grouping")

                at_last = None
                for sub in range(ECH // LCH):
                    lc = ec * ECH + sub * LCH
                    op = psum.tile([P, LCH], F32, tag="op", bufs=2)
                    for lt in range(NLT):
                        mm = nc.tensor.matmul(
                            op, lhsT=vt[:, lt, ds(P * h, P)],
                            rhs=pt_tiles[lt][:, ds(sub * LCH, LCH)],
                            start=(lt == 0), stop=(lt == NLT - 1))
                        if at_last is None and BLOCK_DEPS:
                            add_dep_helper(sc_insts[-1].ins, mm.ins, sync=False,
                                           reason="pe mode grouping")
                        at_last = mm
                    # rows 64..127 are the denominator, already broadcast
                    rs = sb.tile([DHEAD, LCH], F32, tag="rs", bufs=2)
                    nc.vector.reciprocal(rs, op[DHEAD:P, :])
                    nc.vector.tensor_mul(
                        o_norm[ds(64 * h, 64), p, ds(lc, LCH)],
                        op[0:DHEAD, :], rs)
                prev_block_last[0] = at_last

    # ---- out-projection: Y[l, m] = sum_pairs O^T.T @ Wd ----
    for lt in range(NLT):
        for mc in range(NMC):
            yp = psum.tile([P, MCH], F32, tag="qkvp", bufs=2)
            for p in range(NPAIR):
                nc.tensor.matmul(
                    yp, lhsT=o_norm[:, p, ds(lt * P, P)],
                    rhs=wd_sb[:, p, ds(mc * MCH, MCH)],
                    start=(p == 0), stop=(p == NPAIR - 1))
            ys = sb.tile([P, MCH], F32, tag="ys", bufs=3)
            nc.vector.tensor_copy(ys, yp)
            nc.sync.dma_start(y_d[ds(lt * P, P), ds(mc * MCH, MCH)], ys)


_NC_CACHE = {}


def _get_nc():
    if "nc" not in _NC_CACHE:
        _NC_CACHE["nc"] = build_nc()
    return _NC_CACHE["nc"]


def shard_inputs(x, Wq, bq, Wk, bk, Wv, bv, Wd, bd):
    """Build the 8 per-core input maps."""
    in_maps = []
    for c in range(NCORES):
        b = c // (NCORES // B)
        h0 = (c % (NCORES // B)) * H_PER_CORE
        hs = slice(h0, h0 + H_PER_CORE)
        in_maps.append({
            "x": np.ascontiguousarray(np.asarray(x[b], np.float32)),
            "wq": np.ascontiguousarray(np.asarray(Wq[:, hs, :], np.float32).reshape(DMODEL, -1)),
            "wk": np.ascontiguousarray(np.asarray(Wk[:, hs, :], np.float32).reshape(DMODEL, -1)),
            "wv": np.ascontiguousarray(np.asarray(Wv[:, hs, :], np.float32).reshape(DMODEL, -1)),
            "wd": np.ascontiguousarray(np.asarray(Wd[hs], np.float32).reshape(-1, DMODEL)),
            "bq": np.ascontiguousarray(np.asarray(bq[hs], np.float32).reshape(-1)),
            "bk": np.ascontiguousarray(np.asarray(bk[hs], np.float32).reshape(-1)),
            "bv": np.ascontiguousarray(np.asarray(bv[hs], np.float32).reshape(-1)),
        })
    return in_maps


def gather_outputs(results, bd):
    """Sum partial outputs per batch and add bd."""
    out = np.zeros((B, L, DMODEL), np.float32)
    per_b = NCORES // B
    for c, res in enumerate(results):
        out[c // per_b] += res["y"]
    out += np.asarray(bd, np.float32)[None, None, :]
    return out


def kernel(x, Wq, bq, Wk, bk, Wv, bv, Wd, bd, _trace=False):
    nc = _get_nc()
    in_maps = shard_inputs(x, Wq, bq, Wk, bk, Wv, bv, Wd, bd)
    res = run_bass_kernel_spmd(nc, in_maps, list(range(NCORES)), trace=_trace)
    out = gather_outputs(res.results, bd)
    if _trace:
        kernel.last_results = res
    return out
